# revision 4
# baseline (speedup 1.0000x reference)
"""Causal self-attention (RoPE + RMS-norm QK, 16 heads) on 8 Trainium2 cores.

Sharding: core c = (b, g) with b = c // 4 (batch), g = c % 4 (head group of 4).
Each core computes q/k/v projections for its 4 heads from x[b], runs causal
attention, and the out-projection restricted to its head-group columns of
wproj; the host sums the 4 partial outputs per batch.

Per-core layout ("transposed-S"): projections produce Q^T/K^T with head-dim
on partitions (the natural layout for the QK^T contraction), V in natural
[t, d] layout (the natural lhsT for P·V). Scores are computed transposed
(S^T[tk, tq]) so softmax needs no transposes: exp only (logits are bounded
by sqrt(D) after RMS-norm, so max-subtraction is unnecessary), the
denominator comes from an all-ones lhsT matmul that broadcasts column sums
across partitions, and the divide happens on the P·V result's move out of
PSUM. The out-projection is fused per tq-chunk. All heavy matmuls run in
fp32r (hw-rounded fp32, ~1.5e-4 rel err, 4x the fp32 PE rate).
"""

import numpy as np

import concourse.bass as bass
import concourse.mybir as mybir
import concourse.tile as tile
from concourse import bacc
from concourse.bass_utils import run_bass_kernel_spmd

P = 128          # partitions / head dim
T = 2048         # sequence length
C = 2048         # model dim
HL = 4           # heads per core
DL = HL * P      # local projection width (512)
NCO = C // P     # c-chunks (16)
XCH = 256        # x t-chunk width for projections
NXCH = T // XCH  # 8
QCH = 512        # tq-chunk width for attention
NQCH = T // QCH  # 4
NTT = T // P     # t-tiles (16)

F32 = mybir.dt.float32
F32R = mybir.dt.float32r
MUL = mybir.AluOpType.mult


def build_program():
    nc = bacc.Bacc("TRN2", target_bir_lowering=False, debug=False, num_devices=8)

    xT = nc.dram_tensor("xT", [C, T], F32R, kind="ExternalInput")
    wqT = nc.dram_tensor("wqT", [C, DL], F32R, kind="ExternalInput")
    wkT = nc.dram_tensor("wkT", [C, DL], F32R, kind="ExternalInput")
    wvT = nc.dram_tensor("wvT", [C, DL], F32R, kind="ExternalInput")
    wpT = nc.dram_tensor("wpT", [DL, C], F32R, kind="ExternalInput")
    csA_d = nc.dram_tensor("csA", [P, T], F32, kind="ExternalInput")   # cos|cos
    csB_d = nc.dram_tensor("csB", [P, T], F32, kind="ExternalInput")   # sin|sin
    tri_d = nc.dram_tensor("tri", [P, P], F32R, kind="ExternalInput")
    ones_d = nc.dram_tensor("ones", [P, P], F32R, kind="ExternalInput")
    out_p = nc.dram_tensor("out_p", [T, C], F32, kind="ExternalOutput")

    xT_r = xT.ap().rearrange("(co p) t -> p co t", p=P)

    with tile.TileContext(nc) as tc:
        with (
            tc.tile_pool(name="base", bufs=1) as base,
            tc.tile_pool(name="ps_acc", bufs=3, space="PSUM") as ps_acc,
            tc.tile_pool(name="ps_st", bufs=3, space="PSUM") as ps_st,
            tc.tile_pool(name="ps_od", bufs=1, space="PSUM") as ps_od,
        ):
            QT_sb = base.tile([P, HL, T], F32R, tag="QT")   # [d, h, tq]
            KT_sb = base.tile([P, HL, T], F32R, tag="KT")   # [d, h, tk]
            ones_sb = base.tile([P, P], F32R, tag="ones")
            tri_sb = base.tile([P, P], F32R, tag="tri")
            nc.sync.dma_start(ones_sb[:], ones_d.ap())
            nc.sync.dma_start(tri_sb[:], tri_d.ap())

            # ---- phases A/B: Q then K projection + RoPE + RMS-norm ----
            with (
                tc.tile_pool(name="ab", bufs=1) as ab,
                tc.tile_pool(name="abw", bufs=2) as abw,
                tc.tile_pool(name="abx", bufs=2) as abx,
            ):
                csA_sb = ab.tile([P, T], F32, tag="csA")
                csB_sb = ab.tile([P, T], F32, tag="csB")
                nc.sync.dma_start(csA_sb[:], csA_d.ap())
                nc.sync.dma_start(csB_sb[:], csB_d.ap())

                for w_dram, dst_sb, sqrt_scale, name in (
                    (wqT, QT_sb, 1.0, "q"),
                    (wkT, KT_sb, float(P), "k"),
                ):
                    w_sb = ab.tile([P, NCO, DL], F32R, tag="w")
                    nc.sync.dma_start(
                        w_sb[:], w_dram.ap().rearrange("(co p) d -> p co d", p=P)
                    )
                    for tcx in range(NXCH):
                        cols = slice(tcx * XCH, (tcx + 1) * XCH)
                        x_sb = abx.tile([P, NCO, XCH], F32R, tag="x")
                        nc.sync.dma_start(x_sb[:], xT_r[:, :, cols])
                        for h in range(HL):
                            psq = ps_acc.tile([P, XCH], F32, tag="acc")
                            for c in range(NCO):
                                nc.tensor.matmul(
                                    psq[:],
                                    w_sb[:, c, h * P : (h + 1) * P],
                                    x_sb[:, c, :],
                                    start=(c == 0),
                                    stop=(c == NCO - 1),
                                )
                            # RoPE: rows 0:64 = "q1" (d<64), rows 64:128 = "q2"
                            qr = abw.tile([P, XCH], F32, tag="qr")
                            tmp = abw.tile([P, XCH], F32, tag="tmp")
                            lo, hi = slice(0, 64), slice(64, P)
                            nc.vector.tensor_tensor(qr[lo, :], psq[lo, :], csA_sb[lo, cols], MUL)
                            nc.vector.tensor_tensor(tmp[lo, :], psq[hi, :], csB_sb[lo, cols], MUL)
                            nc.vector.tensor_tensor(
                                qr[lo, :], qr[lo, :], tmp[lo, :], mybir.AluOpType.add
                            )
                            nc.vector.tensor_tensor(qr[hi, :], psq[hi, :], csA_sb[hi, cols], MUL)
                            nc.vector.tensor_tensor(tmp[hi, :], psq[lo, :], csB_sb[hi, cols], MUL)
                            nc.vector.tensor_tensor(
                                qr[hi, :], qr[hi, :], tmp[hi, :], mybir.AluOpType.subtract
                            )
                            # RMS: ssq broadcast over partitions via all-ones lhsT
                            q2t = abw.tile([P, XCH], F32R, tag="q2t")
                            nc.vector.tensor_tensor(q2t[:], qr[:], qr[:], MUL)
                            ssq = ps_st.tile([P, XCH], F32, tag="st")
                            nc.tensor.matmul(ssq[:], ones_sb[:], q2t[:], start=True, stop=True)
                            r1 = abw.tile([P, XCH], F32, tag="r1")
                            nc.vector.reciprocal(r1[:], ssq[:])
                            rinv = abw.tile([P, XCH], F32, tag="rinv")
                            # q-side scale=1: rsqrt(ssq) == rsqrt(ms)/sqrt(D); k-side: rsqrt(ms)
                            nc.scalar.activation(
                                rinv[:], r1[:], mybir.ActivationFunctionType.Sqrt,
                                scale=sqrt_scale,
                            )
                            nc.vector.tensor_tensor(dst_sb[:, h, cols], qr[:], rinv[:], MUL)

            with tc.tile_pool(name="vp", bufs=1) as vp:
                V_sb = vp.tile([P, NTT, DL], F32R, tag="V")   # [t_sub, t_tile, d]

                # ---- phase C: V projection ----------------------------
                with (
                    tc.tile_pool(name="cw", bufs=1) as cw,
                    tc.tile_pool(name="cx", bufs=2) as cx,
                ):
                    wv_sb = cw.tile([P, NCO, DL], F32R, tag="wv")
                    nc.sync.dma_start(
                        wv_sb[:], wvT.ap().rearrange("(co p) d -> p co d", p=P)
                    )
                    for tcx in range(NXCH):
                        cols = slice(tcx * XCH, (tcx + 1) * XCH)
                        x_sb = cx.tile([P, NCO, XCH], F32R, tag="x")
                        nc.sync.dma_start(x_sb[:], xT_r[:, :, cols])
                        for m in range(XCH // P):
                            psv = ps_acc.tile([P, DL], F32, tag="acc")
                            for c in range(NCO):
                                nc.tensor.matmul(
                                    psv[:],
                                    x_sb[:, c, m * P : (m + 1) * P],
                                    wv_sb[:, c, :],
                                    start=(c == 0),
                                    stop=(c == NCO - 1),
                                )
                            nc.scalar.copy(V_sb[:, tcx * (XCH // P) + m, :], psv[:])

                # ---- phase D: attention + fused out-projection --------
                with (
                    tc.tile_pool(name="dw", bufs=1) as dw,
                    tc.tile_pool(name="de", bufs=6) as de,
                    tc.tile_pool(name="dm", bufs=2) as dm,
                ):
                    wp_sb = dw.tile([P, HL, C], F32R, tag="wp")
                    nc.sync.dma_start(
                        wp_sb[:], wpT.ap().rearrange("(h p) j -> p h j", p=P)
                    )
                    for j in range(NQCH):
                        ot_ch = dm.tile([P, HL, QCH], F32R, tag="otch")
                        ntk = (j + 1) * (QCH // P)
                        for h in range(HL):
                            ot_ps = ps_od.tile([P, QCH], F32, tag="ot")
                            den_ps = ps_od.tile([P, QCH], F32, tag="den")
                            for i in range(ntk):
                                r = i - j * (QCH // P)
                                f0 = r * P if r >= 0 else 0
                                ecols = slice(f0, QCH)
                                st = ps_st.tile([P, QCH], F32, tag="st")
                                nc.tensor.matmul(
                                    st[:, ecols],
                                    KT_sb[:, h, i * P : (i + 1) * P],
                                    QT_sb[:, h, j * QCH + f0 : (j + 1) * QCH],
                                    start=True,
                                    stop=True,
                                )
                                e_sb = de.tile([P, QCH], F32R, tag="e")
                                nc.scalar.activation(
                                    e_sb[:, ecols], st[:, ecols],
                                    mybir.ActivationFunctionType.Exp,
                                )
                                if r >= 0:
                                    nc.vector.tensor_tensor(
                                        e_sb[:, f0 : f0 + P], e_sb[:, f0 : f0 + P],
                                        tri_sb[:], MUL,
                                    )
                                nc.tensor.matmul(
                                    ot_ps[:, ecols],
                                    V_sb[:, i, h * P : (h + 1) * P],
                                    e_sb[:, ecols],
                                    start=(i == 0),
                                    stop=(i == ntk - 1),
                                )
                                nc.tensor.matmul(
                                    den_ps[:, ecols],
                                    ones_sb[:],
                                    e_sb[:, ecols],
                                    start=(i == 0),
                                    stop=(i == ntk - 1),
                                )
                            recip = dm.tile([P, QCH], F32, tag="recip")
                            nc.vector.reciprocal(recip[:], den_ps[:])
                            nc.vector.tensor_tensor(
                                ot_ch[:, h, :], ot_ps[:], recip[:], MUL
                            )
                        # fused out-projection for tq-chunk j
                        for u in range(QCH // P):
                            for jc in range(NQCH):
                                po = ps_acc.tile([P, QCH], F32, tag="acc")
                                for h in range(HL):
                                    nc.tensor.matmul(
                                        po[:],
                                        ot_ch[:, h, u * P : (u + 1) * P],
                                        wp_sb[:, h, jc * QCH : (jc + 1) * QCH],
                                        start=(h == 0),
                                        stop=(h == HL - 1),
                                    )
                                osb = dm.tile([P, QCH], F32, tag="osb")
                                nc.scalar.copy(osb[:], po[:])
                                nc.sync.dma_start(
                                    out_p.ap()[
                                        j * QCH + u * P : j * QCH + (u + 1) * P,
                                        jc * QCH : (jc + 1) * QCH,
                                    ],
                                    osb[:],
                                )

    nc.compile()
    return nc


_NC = None


def _get_nc():
    global _NC
    if _NC is None:
        _NC = build_program()
    return _NC


def _host_inputs(x, cos, sin, wq, wk, wv, wproj):
    B = x.shape[0]
    cosT = np.ascontiguousarray(cos[0, :, 0, :].T).astype(np.float32)  # [64, T]
    sinT = np.ascontiguousarray(sin[0, :, 0, :].T).astype(np.float32)
    csA = np.concatenate([cosT, cosT], axis=0)
    csB = np.concatenate([sinT, sinT], axis=0)
    tri = np.triu(np.ones((P, P), np.float32))
    ones = np.ones((P, P), np.float32)

    xTs = [np.ascontiguousarray(x[b].T) for b in range(B)]
    in_maps = []
    for core in range(8):
        b, g = divmod(core, 4)
        sl = slice(g * DL, (g + 1) * DL)
        in_maps.append({
            "xT": xTs[b],
            "wqT": np.ascontiguousarray(wq[sl, :].T),
            "wkT": np.ascontiguousarray(wk[sl, :].T),
            "wvT": np.ascontiguousarray(wv[sl, :].T),
            "wpT": np.ascontiguousarray(wproj[:, sl].T),
            "csA": csA, "csB": csB, "tri": tri, "ones": ones,
        })
    return in_maps


def kernel(x, cos, sin, wq, wk, wv, wproj, _trace=False):
    nc = _get_nc()
    in_maps = _host_inputs(x, cos, sin, wq, wk, wv, wproj)
    res = run_bass_kernel_spmd(nc, in_maps, core_ids=list(range(8)), trace=_trace)
    parts = [res.results[c]["out_p"].astype(np.float64) for c in range(8)]
    out = np.stack([
        sum(parts[0:4]).astype(np.float32),
        sum(parts[4:8]).astype(np.float32),
    ])
    kernel.last_exec_time_ns = res.exec_time_ns
    return out
